# revision 4
# baseline (speedup 1.0000x reference)
"""Trainium2 Bass kernel for nn_InteractionSitesModule.

Math (per sample b):
  pw_c  = gelu(x_c @ W_pc + b_pc) * cmask[:,None]    [128,64]
  pw_p  = gelu(x_p @ W_pp + b_pp) * pmask[:,None]    [1024,64]
  lat_c = gelu(x_c @ W_lc + b_lc) * cmask[:,None]
  lat_p = gelu(x_p @ W_lp + b_lp) * pmask[:,None]
  mmap  = pw_c @ pw_p.T                              [128,1024]
  P     = mmap * pairwise_mask
  row   = P.sum(1); col = P.sum(0)
  lv    = [row @ lat_c, col @ lat_p]                 [128]
  pair_out = sigmoid(mmap) * outer(cmask, pmask)

Sharding: data-parallel over batch B=32 across 8 cores (4 samples/core).

Layout strategy per core:
  - x is loaded in natural layout [l,768] and transposed on the PE
    (fp32 transpose via identity; exact) into xT [768,l] tiles so the
    768-dim contraction sits on partitions.
  - The two 768->64 linears per input share one matmul: W = [W_pw|W_lat]
    [768,128], so proj^T = W^T @ x^T comes out as [128(f_pw|f_lat), l].
  - Projection / pairwise matmuls run as float32r (fast fp32 mode,
    ~1.5e-4 rel err); rounding to f32r is folded into copies/mask-muls.
  - col sums run on the PE as P_slice^T @ ones -> [128,1] per j-tile,
    which lands column sums partition-major for the lv_p matmul.
"""
import numpy as np

HID = 768
PROJ = 64
B, Lc, Lp = 32, 128, 1024
NCORES = 8
BPC = B // NCORES  # samples per core
KT = HID // 128    # 6 contraction tiles
LPC = 512          # protein l-chunk
NPC = Lp // LPC    # 2 protein chunks per sample


# ---------------------------------------------------------------------------
# walrus in this image caps sync-waits at 1 per instruction; spill extras
# onto same-engine NoOps inserted immediately before the instruction.
def _spill_excess_waits(nc):
    import bass_rust
    import concourse.mybir as mybir

    n = 0
    for f in nc.m.functions:
        for bb in f.blocks:
            insts = bb.instructions
            i = 0
            while i < len(insts):
                inst = insts[i]
                si = inst.sync_info
                waits = list(si.on_wait) if si and si.on_wait else []
                if len(waits) > 1:
                    si.on_wait = waits[:1]
                    for w in waits[1:]:
                        n += 1
                        nop = mybir.InstNoOp(name=f"I-wspill-{n}", ins=[], outs=[])
                        nop.engine = inst.engine
                        nop.sync_info = bass_rust.SyncInfo(on_wait=[w], on_update=[])
                        insts.insert(i, nop)
                        i += 1
                i += 1
    return n


def _build(nc):
    import concourse.tile as tile
    import concourse.mybir as mybir

    F32 = mybir.dt.float32
    F32R = mybir.dt.float32r
    AF = mybir.ActivationFunctionType

    xp_d = nc.declare_dram_parameter("xp", [BPC, Lp, HID], F32, isOutput=False)
    xc_d = nc.declare_dram_parameter("xc", [BPC, Lc, HID], F32, isOutput=False)
    pm_d = nc.declare_dram_parameter("pm", [BPC, Lp], F32, isOutput=False)
    cm_d = nc.declare_dram_parameter("cm", [BPC, Lc], F32, isOutput=False)
    pwm_d = nc.declare_dram_parameter("pwm", [BPC, Lc, Lp], F32, isOutput=False)
    wc_d = nc.declare_dram_parameter("wc", [HID, 128], F32, isOutput=False)
    wp_d = nc.declare_dram_parameter("wp", [HID, 128], F32, isOutput=False)
    bc_d = nc.declare_dram_parameter("bc", [128], F32, isOutput=False)
    bp_d = nc.declare_dram_parameter("bp", [128], F32, isOutput=False)
    id_d = nc.declare_dram_parameter("ident", [128, 128], F32, isOutput=False)
    opair_d = nc.declare_dram_parameter("opair", [BPC, Lc, Lp], F32, isOutput=True)
    olat_d = nc.declare_dram_parameter("olat", [BPC, 2 * PROJ], F32, isOutput=True)

    with tile.TileContext(nc) as tc:
        with tc.tile_pool(name="cst", bufs=1) as cst, \
             tc.tile_pool(name="sb", bufs=2) as sb, \
             tc.tile_pool(name="ps2", bufs=2, space="PSUM") as ps2, \
             tc.tile_pool(name="ps1", bufs=1, space="PSUM") as ps1:

            # ---- one-time constants ----
            ident = cst.tile([128, 128], F32, tag="ident")
            nc.sync.dma_start(ident[:], id_d[:])
            ident_r = cst.tile([128, 128], F32R, tag="identr")
            nc.vector.tensor_copy(ident_r[:], ident[:])
            ones_col = cst.tile([128, 1], F32, tag="ones")
            nc.vector.memset(ones_col[:], 1.0)

            w_r = {}
            bias = {}
            for key, wd, bd in (("c", wc_d, bc_d), ("p", wp_d, bp_d)):
                wt = cst.tile([128, KT, 128], F32, tag=f"w{key}")
                nc.sync.dma_start(wt[:], wd.rearrange("(t p) f -> p t f", p=128))
                wr = cst.tile([128, KT, 128], F32R, tag=f"w{key}r")
                nc.vector.tensor_copy(wr[:], wt[:])
                w_r[key] = wr
                bt = cst.tile([128, 1], F32, tag=f"b{key}")
                nc.sync.dma_start(bt[:], bd[:, None])
                bias[key] = bt

            # staging for the four latent vectors [1, BPC*128]
            lat_stage = cst.tile([1, BPC * 128], F32, tag="latstage")

            for b in range(BPC):
                # ---- per-sample mask tiles ----
                pmb = sb.tile([128, Lp], F32, tag="pmb")
                nc.sync.dma_start(pmb[:], pm_d[b][None, :].partition_broadcast(128))
                cmb = sb.tile([128, Lc], F32, tag="cmb")
                nc.sync.dma_start(cmb[:], cm_d[b][None, :].partition_broadcast(128))
                cm_col = sb.tile([128, 1], F32, tag="cmcol")
                nc.sync.dma_start(cm_col[:], cm_d[b][:, None])
                pwm_sb = sb.tile([128, Lp], F32, tag="pwm")
                nc.sync.dma_start(pwm_sb[:], pwm_d[b])

                # ---- protein projections: projm_p [128(f_pw|f_lat), 1024] f32r
                projm_p = sb.tile([128, Lp], F32R, tag="projmp")
                for ch in range(NPC):
                    l0 = ch * LPC
                    xn = sb.tile([128, 4, HID], F32, tag="xnat")
                    nc.sync.dma_start(
                        xn[:], xp_d[b].rearrange("(c q p) d -> p c q d", p=128, q=4)[:, ch, :, :])
                    xt = sb.tile([128, KT, LPC], F32R, tag="xt")
                    for k in range(KT):
                        tp = ps2.tile([128, LPC], F32, tag="tp")
                        for q in range(4):
                            nc.tensor.transpose(
                                tp[:, q * 128:(q + 1) * 128],
                                xn[:, q, k * 128:(k + 1) * 128], ident[:])
                        if k % 2 == 0:
                            nc.vector.tensor_copy(xt[:, k, :], tp[:])
                        else:
                            nc.scalar.copy(xt[:, k, :], tp[:])
                    pj = ps2.tile([128, LPC], F32, tag="proj")
                    for k in range(KT):
                        nc.tensor.matmul(pj[:], w_r["p"][:, k, :], xt[:, k, :],
                                         start=(k == 0), stop=(k == KT - 1))
                    gl = sb.tile([128, LPC], F32, tag="gelu")
                    nc.scalar.activation(gl[:], pj[:], AF.Gelu, bias=bias["p"][:])
                    nc.vector.tensor_mul(projm_p[:, l0:l0 + LPC], gl[:], pmb[:, l0:l0 + LPC])

                # ---- compound projections: projm_c [128, 128] f32r
                xnc = sb.tile([128, HID], F32, tag="xnatc")
                nc.sync.dma_start(xnc[:], xc_d[b])
                xtc = sb.tile([128, KT, Lc], F32R, tag="xtc")
                for k in range(KT):
                    tpc = ps2.tile([128, Lc], F32, tag="tp")
                    nc.tensor.transpose(tpc[:], xnc[:, k * 128:(k + 1) * 128], ident[:])
                    if k % 2 == 0:
                        nc.vector.tensor_copy(xtc[:, k, :], tpc[:])
                    else:
                        nc.scalar.copy(xtc[:, k, :], tpc[:])
                pjc = ps2.tile([128, Lc], F32, tag="proj")
                for k in range(KT):
                    nc.tensor.matmul(pjc[:], w_r["c"][:, k, :], xtc[:, k, :],
                                     start=(k == 0), stop=(k == KT - 1))
                glc = sb.tile([128, Lc], F32, tag="geluc")
                nc.scalar.activation(glc[:], pjc[:], AF.Gelu, bias=bias["c"][:])
                projm_c = sb.tile([128, Lc], F32R, tag="projmc")
                nc.vector.tensor_mul(projm_c[:], glc[:], cmb[:])

                # ---- transpose lat projections back to natural layout (f32)
                lat_c = sb.tile([128, PROJ], F32, tag="latc")
                ltp = ps2.tile([128, 128], F32R, tag="tp")
                nc.tensor.transpose(ltp[:], projm_c[:], ident_r[:])
                nc.vector.tensor_copy(lat_c[:], ltp[:, PROJ:128])
                lat_p = sb.tile([128, 8, PROJ], F32, tag="latp")
                for t in range(8):
                    ltt = ps2.tile([128, 128], F32R, tag="tp")
                    nc.tensor.transpose(
                        ltt[:], projm_p[:, t * 128:(t + 1) * 128], ident_r[:])
                    nc.scalar.copy(lat_p[:, t, :], ltt[:, PROJ:128])

                # ---- pairwise map, sigmoid path, P path
                P_sb = sb.tile([128, Lp], F32, tag="P")
                outp = sb.tile([128, Lp], F32, tag="outp")
                for ch in range(NPC):
                    l0 = ch * LPC
                    mm = ps2.tile([128, LPC], F32, tag="mm")
                    nc.tensor.matmul(mm[:], projm_c[0:PROJ, :], projm_p[0:PROJ, l0:l0 + LPC])
                    sg = sb.tile([128, LPC], F32, tag="sig")
                    nc.scalar.activation(sg[:], mm[:], AF.Sigmoid)
                    nc.vector.tensor_mul(P_sb[:, l0:l0 + LPC], mm[:], pwm_sb[:, l0:l0 + LPC])
                    nc.vector.tensor_mul(sg[:], sg[:], pmb[:, l0:l0 + LPC])
                    nc.vector.tensor_scalar_mul(outp[:, l0:l0 + LPC], sg[:], cm_col[:])
                nc.sync.dma_start(opair_d[b], outp[:])

                # ---- reductions ----
                row = sb.tile([128, 1], F32, tag="row")
                nc.vector.reduce_sum(row[:], P_sb[:], axis=mybir.AxisListType.X)
                colt_ps = ps1.tile([128, 8], F32, tag="colt")
                for t in range(8):
                    nc.tensor.matmul(colt_ps[:, t:t + 1],
                                     P_sb[:, t * 128:(t + 1) * 128], ones_col[:])
                colt = sb.tile([128, 8], F32, tag="coltsb")
                nc.vector.tensor_copy(colt[:], colt_ps[:])

                lv = ps1.tile([1, 2 * PROJ], F32, tag="lv")
                nc.tensor.matmul(lv[:, 0:PROJ], row[:], lat_c[:])
                for t in range(8):
                    nc.tensor.matmul(lv[:, PROJ:2 * PROJ], colt[:, t:t + 1],
                                     lat_p[:, t, :], start=(t == 0), stop=(t == 7))
                nc.vector.tensor_copy(lat_stage[:, b * 128:(b + 1) * 128], lv[:])

            nc.sync.dma_start(olat_d.rearrange("b f -> (b f)")[None, :], lat_stage[:])

    _spill_excess_waits(nc)
    return nc


_NC_CACHE = [None]


def kernel(protein_features, pocket_mask, compound_features, compound_mask,
           pairwise_mask, W_pc, b_pc, W_pp, b_pp, W_lc, b_lc, W_lp, b_lp):
    import concourse.bass as bass
    from concourse.bass_utils import run_bass_kernel_spmd

    if _NC_CACHE[0] is None:
        _NC_CACHE[0] = _build(bass.Bass())
    nc = _NC_CACHE[0]

    f32 = np.float32
    wc = np.ascontiguousarray(np.concatenate([W_pc, W_lc], axis=1), dtype=f32)
    wp = np.ascontiguousarray(np.concatenate([W_pp, W_lp], axis=1), dtype=f32)
    bc = np.ascontiguousarray(np.concatenate([b_pc, b_lc]), dtype=f32)
    bp = np.ascontiguousarray(np.concatenate([b_pp, b_lp]), dtype=f32)
    ident = np.eye(128, dtype=f32)

    in_maps = []
    for c in range(NCORES):
        s = slice(c * BPC, (c + 1) * BPC)
        in_maps.append({
            "xp": np.ascontiguousarray(protein_features[s], dtype=f32),
            "xc": np.ascontiguousarray(compound_features[s], dtype=f32),
            "pm": np.ascontiguousarray(pocket_mask[s], dtype=f32),
            "cm": np.ascontiguousarray(compound_mask[s], dtype=f32),
            "pwm": np.ascontiguousarray(pairwise_mask[s], dtype=f32),
            "wc": wc, "wp": wp, "bc": bc, "bp": bp, "ident": ident,
        })

    res = run_bass_kernel_spmd(nc, in_maps, core_ids=list(range(NCORES)))
    latent = np.concatenate([res.results[c]["olat"] for c in range(NCORES)], axis=0)
    pair = np.concatenate([res.results[c]["opair"] for c in range(NCORES)], axis=0)
    return latent, pair


# revision 7
# speedup vs baseline: 1.1536x; 1.1536x over previous
"""Trainium2 Bass kernel for nn_InteractionSitesModule.

Math (per sample b):
  pw_c  = gelu(x_c @ W_pc + b_pc) * cmask[:,None]    [128,64]
  pw_p  = gelu(x_p @ W_pp + b_pp) * pmask[:,None]    [1024,64]
  lat_c = gelu(x_c @ W_lc + b_lc) * cmask[:,None]
  lat_p = gelu(x_p @ W_lp + b_lp) * pmask[:,None]
  mmap  = pw_c @ pw_p.T                              [128,1024]
  P     = mmap * pairwise_mask
  row   = P.sum(1); col = P.sum(0)
  lv    = [row @ lat_c, col @ lat_p]                 [128]
  pair_out = sigmoid(mmap) * outer(cmask, pmask)

Sharding: data-parallel over batch B=32 across 8 cores (4 samples/core).

Layout strategy per core:
  - x is cast-loaded (SWDGE) as float32r in natural layout [l,768] and
    transposed on the PE (f32r transpose, pure data movement) into
    xT [768,l] tiles so the 768-dim contraction sits on partitions.
  - The two 768->64 linears per input share one matmul: W = [W_pw|W_lat]
    [768,128], so proj^T = W^T @ x^T comes out as [128(f_pw|f_lat), l].
  - All 4 samples' compound parts are batched into one 512-wide chunk.
  - Projection / pairwise / reduction matmuls run as float32r (fast fp32
    mode, ~1e-4..1e-3 rel err); f32r rounding rides existing copies/muls.
  - col sums run on the PE as P_slice^T @ ones -> [128,1] per j-tile,
    landing column sums partition-major for the lv_p matmul.
"""
import numpy as np

HID = 768
PROJ = 64
B, Lc, Lp = 32, 128, 1024
NCORES = 8
BPC = B // NCORES  # samples per core
KT = HID // 128    # 6 contraction tiles
LPC = 512          # protein l-chunk
NPC = Lp // LPC    # 2 protein chunks per sample


# ---------------------------------------------------------------------------
# walrus in this image caps sync-waits at 1 per instruction; spill extras
# onto same-engine NoOps inserted immediately before the instruction.
def _spill_excess_waits(nc):
    import bass_rust
    import concourse.mybir as mybir

    n = 0
    for f in nc.m.functions:
        for bb in f.blocks:
            insts = bb.instructions
            i = 0
            while i < len(insts):
                inst = insts[i]
                si = inst.sync_info
                waits = list(si.on_wait) if si and si.on_wait else []
                if len(waits) > 1:
                    si.on_wait = waits[:1]
                    for w in waits[1:]:
                        n += 1
                        nop = mybir.InstNoOp(name=f"I-wspill-{n}", ins=[], outs=[])
                        nop.engine = inst.engine
                        nop.sync_info = bass_rust.SyncInfo(on_wait=[w], on_update=[])
                        insts.insert(i, nop)
                        i += 1
                i += 1
    return n


def _build(nc):
    import concourse.tile as tile
    import concourse.mybir as mybir

    F32 = mybir.dt.float32
    F32R = mybir.dt.float32r
    AF = mybir.ActivationFunctionType

    xp_d = nc.declare_dram_parameter("xp", [BPC, Lp, HID], F32, isOutput=False)
    xc_d = nc.declare_dram_parameter("xc", [BPC, Lc, HID], F32, isOutput=False)
    pm_d = nc.declare_dram_parameter("pm", [BPC, Lp], F32, isOutput=False)
    cm_d = nc.declare_dram_parameter("cm", [BPC, Lc], F32, isOutput=False)
    pwm_d = nc.declare_dram_parameter("pwm", [BPC, Lc, Lp], F32, isOutput=False)
    wc_d = nc.declare_dram_parameter("wc", [HID, 128], F32, isOutput=False)
    wp_d = nc.declare_dram_parameter("wp", [HID, 128], F32, isOutput=False)
    bc_d = nc.declare_dram_parameter("bc", [128], F32, isOutput=False)
    bp_d = nc.declare_dram_parameter("bp", [128], F32, isOutput=False)
    id_d = nc.declare_dram_parameter("ident", [128, 128], F32, isOutput=False)
    opair_d = nc.declare_dram_parameter("opair", [BPC, Lc, Lp], F32, isOutput=True)
    olat_d = nc.declare_dram_parameter("olat", [BPC, 2 * PROJ], F32, isOutput=True)

    with tile.TileContext(nc) as tc:
        with tc.tile_pool(name="cst", bufs=1) as cst, \
             tc.tile_pool(name="sb", bufs=2) as sb, \
             tc.tile_pool(name="sb3", bufs=3) as sb3, \
             tc.tile_pool(name="ps3", bufs=3, space="PSUM") as ps3, \
             tc.tile_pool(name="ps2", bufs=2, space="PSUM") as ps2, \
             tc.tile_pool(name="ps1", bufs=1, space="PSUM") as ps1:

            # ---- one-time constants ----
            ident_r = cst.tile([128, 128], F32R, tag="identr")
            nc.gpsimd.dma_start(ident_r[:], id_d[:])
            ones_col = cst.tile([128, 1], F32, tag="ones")
            nc.vector.memset(ones_col[:], 1.0)

            w_r = {}
            bias = {}
            for key, wd, bd in (("c", wc_d, bc_d), ("p", wp_d, bp_d)):
                wr = cst.tile([128, KT, 128], F32R, tag=f"w{key}r")
                nc.gpsimd.dma_start(wr[:], wd.rearrange("(t p) f -> p t f", p=128))
                w_r[key] = wr
                bt = cst.tile([128, 1], F32, tag=f"b{key}")
                nc.sync.dma_start(bt[:], bd[:, None])
                bias[key] = bt

            # staging for the four latent vectors [1, BPC*128]
            lat_stage = cst.tile([1, BPC * 128], F32, tag="latstage")

            def transpose_chunk(xsrc_ap, n_sub, xt_tile, cw):
                """xsrc_ap: f32r natural tiles [128, n_sub, HID]; write
                xT d-tiles [128, KT, n_sub*128] via PE + rounding copies."""
                for k in range(KT):
                    tp = ps3.tile([128, n_sub * 128], F32R, tag="tp")
                    for q in range(n_sub):
                        nc.tensor.transpose(
                            tp[:, q * 128:(q + 1) * 128],
                            xsrc_ap[:, q, k * 128:(k + 1) * 128], ident_r[:])
                    if cw[0] % 2 == 0:
                        nc.vector.tensor_copy(xt_tile[:, k, :], tp[:])
                    else:
                        nc.scalar.copy(xt_tile[:, k, :], tp[:])
                    cw[0] += 1

            cw = [0]  # round-robin DVE/ACT for PSUM->SBUF copies

            # ---- compound projections, all 4 samples batched: [128, 512]
            xnc = sb.tile([128, BPC, HID], F32R, tag="xnatc")
            nc.gpsimd.dma_start(
                xnc[:], xc_d.rearrange("b p d -> p b d"))
            xtc = sb.tile([128, KT, BPC * Lc], F32R, tag="xtc")
            transpose_chunk(xnc, BPC, xtc, cw)
            pjc = ps2.tile([128, BPC * Lc], F32, tag="proj")
            for k in range(KT):
                nc.tensor.matmul(pjc[:], w_r["c"][:, k, :], xtc[:, k, :],
                                 start=(k == 0), stop=(k == KT - 1))
            glc = sb.tile([128, BPC * Lc], F32, tag="geluc")
            nc.scalar.activation(glc[:], pjc[:], AF.Gelu, bias=bias["c"][:])
            cmb = sb.tile([128, BPC * Lc], F32, tag="cmb")
            nc.sync.dma_start(
                cmb[:], cm_d.rearrange("b l -> (b l)")[None, :].partition_broadcast(128))
            projm_c = sb.tile([128, BPC * Lc], F32R, tag="projmc")
            nc.vector.tensor_mul(projm_c[:], glc[:], cmb[:])

            # lat_c natural [128, 64] per sample
            lat_c = sb.tile([128, BPC, PROJ], F32, tag="latc")
            for b in range(BPC):
                ltp = ps3.tile([128, 128], F32R, tag="tp")
                nc.tensor.transpose(
                    ltp[:], projm_c[:, b * 128:(b + 1) * 128], ident_r[:])
                nc.vector.tensor_copy(lat_c[:, b, :], ltp[:, PROJ:128])

            for b in range(BPC):
                # ---- per-sample mask tiles ----
                pmb = sb.tile([128, Lp], F32, tag="pmb")
                nc.sync.dma_start(pmb[:], pm_d[b][None, :].partition_broadcast(128))
                cm_col = sb.tile([128, 1], F32, tag="cmcol")
                nc.sync.dma_start(cm_col[:], cm_d[b][:, None])
                pwm_sb = sb.tile([128, Lp], F32, tag="pwm")
                nc.sync.dma_start(pwm_sb[:], pwm_d[b])

                # ---- protein projections: projm_p [128(f_pw|f_lat), 1024]
                projm_p = sb.tile([128, Lp], F32R, tag="projmp")
                for ch in range(NPC):
                    l0 = ch * LPC
                    xn = sb3.tile([128, 4, HID], F32R, tag="xnat")
                    nc.gpsimd.dma_start(
                        xn[:], xp_d[b].rearrange("(c q p) d -> p c q d", p=128, q=4)[:, ch, :, :])
                    xt = sb3.tile([128, KT, LPC], F32R, tag="xt")
                    transpose_chunk(xn, 4, xt, cw)
                    pj = ps2.tile([128, LPC], F32, tag="proj")
                    for k in range(KT):
                        nc.tensor.matmul(pj[:], w_r["p"][:, k, :], xt[:, k, :],
                                         start=(k == 0), stop=(k == KT - 1))
                    gl = sb.tile([128, LPC], F32, tag="gelu")
                    nc.scalar.activation(gl[:], pj[:], AF.Gelu, bias=bias["p"][:])
                    nc.vector.tensor_mul(projm_p[:, l0:l0 + LPC], gl[:], pmb[:, l0:l0 + LPC])

                # ---- lat_p natural [128, 8, 64]
                lat_p = sb.tile([128, 8, PROJ], F32, tag="latp")
                for t in range(8):
                    ltt = ps3.tile([128, 128], F32R, tag="tp")
                    nc.tensor.transpose(
                        ltt[:], projm_p[:, t * 128:(t + 1) * 128], ident_r[:])
                    nc.scalar.copy(lat_p[:, t, :], ltt[:, PROJ:128])

                # ---- pairwise map, sigmoid path, P path
                P_sb = sb.tile([128, Lp], F32, tag="P")
                outp = sb.tile([128, Lp], F32, tag="outp")
                for ch in range(NPC):
                    l0 = ch * LPC
                    mm = ps2.tile([128, LPC], F32, tag="mm")
                    nc.tensor.matmul(mm[:], projm_c[0:PROJ, b * 128:(b + 1) * 128],
                                     projm_p[0:PROJ, l0:l0 + LPC])
                    sg = sb.tile([128, LPC], F32, tag="sig")
                    nc.scalar.activation(sg[:], mm[:], AF.Sigmoid)
                    nc.vector.tensor_mul(P_sb[:, l0:l0 + LPC], mm[:], pwm_sb[:, l0:l0 + LPC])
                    nc.vector.tensor_mul(sg[:], sg[:], pmb[:, l0:l0 + LPC])
                    nc.vector.tensor_scalar_mul(outp[:, l0:l0 + LPC], sg[:], cm_col[:])
                nc.sync.dma_start(opair_d[b], outp[:])

                # ---- reductions ----
                row = sb.tile([128, 1], F32, tag="row")
                nc.vector.reduce_sum(row[:], P_sb[:], axis=mybir.AxisListType.X)
                colt_ps = ps1.tile([128, 8], F32, tag="small")
                for t in range(8):
                    nc.tensor.matmul(colt_ps[:, t:t + 1],
                                     P_sb[:, t * 128:(t + 1) * 128], ones_col[:])
                colt = sb.tile([128, 8], F32, tag="coltsb")
                nc.vector.tensor_copy(colt[:], colt_ps[:])

                lv = ps1.tile([1, 2 * PROJ], F32, tag="small")
                nc.tensor.matmul(lv[:, 0:PROJ], row[:], lat_c[:, b, :])
                for t in range(8):
                    nc.tensor.matmul(lv[:, PROJ:2 * PROJ], colt[:, t:t + 1],
                                     lat_p[:, t, :], start=(t == 0), stop=(t == 7))
                nc.vector.tensor_copy(lat_stage[:, b * 128:(b + 1) * 128], lv[:])

            nc.sync.dma_start(olat_d.rearrange("b f -> (b f)")[None, :], lat_stage[:])

    _spill_excess_waits(nc)
    return nc


_NC_CACHE = [None]


def kernel(protein_features, pocket_mask, compound_features, compound_mask,
           pairwise_mask, W_pc, b_pc, W_pp, b_pp, W_lc, b_lc, W_lp, b_lp):
    import concourse.bass as bass
    from concourse.bass_utils import run_bass_kernel_spmd

    if _NC_CACHE[0] is None:
        _NC_CACHE[0] = _build(bass.Bass())
    nc = _NC_CACHE[0]

    f32 = np.float32
    wc = np.ascontiguousarray(np.concatenate([W_pc, W_lc], axis=1), dtype=f32)
    wp = np.ascontiguousarray(np.concatenate([W_pp, W_lp], axis=1), dtype=f32)
    bc = np.ascontiguousarray(np.concatenate([b_pc, b_lc]), dtype=f32)
    bp = np.ascontiguousarray(np.concatenate([b_pp, b_lp]), dtype=f32)
    ident = np.eye(128, dtype=f32)

    in_maps = []
    for c in range(NCORES):
        s = slice(c * BPC, (c + 1) * BPC)
        in_maps.append({
            "xp": np.ascontiguousarray(protein_features[s], dtype=f32),
            "xc": np.ascontiguousarray(compound_features[s], dtype=f32),
            "pm": np.ascontiguousarray(pocket_mask[s], dtype=f32),
            "cm": np.ascontiguousarray(compound_mask[s], dtype=f32),
            "pwm": np.ascontiguousarray(pairwise_mask[s], dtype=f32),
            "wc": wc, "wp": wp, "bc": bc, "bp": bp, "ident": ident,
        })

    res = run_bass_kernel_spmd(nc, in_maps, core_ids=list(range(NCORES)))
    latent = np.concatenate([res.results[c]["olat"] for c in range(NCORES)], axis=0)
    pair = np.concatenate([res.results[c]["opair"] for c in range(NCORES)], axis=0)
    return latent, pair


# revision 14
# speedup vs baseline: 1.5074x; 1.3067x over previous
"""Trainium2 Bass kernel for nn_InteractionSitesModule.

Math (per sample b):
  pw_c  = gelu(x_c @ W_pc + b_pc) * cmask[:,None]    [128,64]
  pw_p  = gelu(x_p @ W_pp + b_pp) * pmask[:,None]    [1024,64]
  lat_c = gelu(x_c @ W_lc + b_lc) * cmask[:,None]
  lat_p = gelu(x_p @ W_lp + b_lp) * pmask[:,None]
  mmap  = pw_c @ pw_p.T                              [128,1024]
  P     = mmap * pairwise_mask
  row   = P.sum(1); col = P.sum(0)
  lv    = [row @ lat_c, col @ lat_p]                 [128]
  pair_out = sigmoid(mmap) * outer(cmask, pmask)

Sharding: data-parallel over batch B=32 across 8 cores (4 samples/core).

Layout strategy per core:
  - x is cast-loaded (SWDGE) as float32r in natural layout [l,768] and
    transposed on the PE (f32r transpose, pure data movement) into
    xT [768,l] tiles so the 768-dim contraction sits on partitions.
  - The two 768->64 linears per input share one matmul: W = [W_pw|W_lat]
    [768,128], so proj^T = W^T @ x^T comes out as [128(f_pw|f_lat), l].
  - All 4 samples' compound parts are batched into one 512-wide chunk.
  - Projection / pairwise matmuls run as float32r (fast fp32 mode,
    ~1e-4..1e-3 rel err); f32r rounding rides existing copies/muls.
  - Two passes over samples: pass 1 = projections (Gelu only on ScalarE),
    pass 2 = pairwise/sigmoid/reductions, minimizing ACT table reloads.
  - Row-vector masks are broadcast across partitions on GpSimd, not DMA.
  - col sums run on the PE as P_slice^T @ ones -> [128,1] per j-tile,
    landing column sums partition-major for the lv_p matmul.
"""
import numpy as np

HID = 768
PROJ = 64
B, Lc, Lp = 32, 128, 1024
NCORES = 8
BPC = B // NCORES  # samples per core
KT = HID // 128    # 6 contraction tiles
LPC = 512          # protein l-chunk
NPC = Lp // LPC    # 2 protein chunks per sample


# ---------------------------------------------------------------------------
# walrus in this image caps sync-waits at 1 per instruction; spill extras
# onto same-engine NoOps inserted immediately before the instruction.
def _spill_excess_waits(nc):
    import bass_rust
    import concourse.mybir as mybir

    n = 0
    for f in nc.m.functions:
        for bb in f.blocks:
            insts = bb.instructions
            i = 0
            while i < len(insts):
                inst = insts[i]
                si = inst.sync_info
                waits = list(si.on_wait) if si and si.on_wait else []
                if len(waits) > 1:
                    si.on_wait = waits[:1]
                    for w in waits[1:]:
                        n += 1
                        nop = mybir.InstNoOp(name=f"I-wspill-{n}", ins=[], outs=[])
                        nop.engine = inst.engine
                        nop.sync_info = bass_rust.SyncInfo(on_wait=[w], on_update=[])
                        insts.insert(i, nop)
                        i += 1
                i += 1
    return n


def _build(nc):
    import concourse.tile as tile
    import concourse.mybir as mybir

    F32 = mybir.dt.float32
    F16 = mybir.dt.float16
    AF = mybir.ActivationFunctionType

    xp_d = nc.declare_dram_parameter("xp", [BPC, Lp, HID], F32, isOutput=False)
    xc_d = nc.declare_dram_parameter("xc", [BPC, Lc, HID], F32, isOutput=False)
    pm_d = nc.declare_dram_parameter("pm", [BPC, Lp], F32, isOutput=False)
    cm_d = nc.declare_dram_parameter("cm", [BPC, Lc], F32, isOutput=False)
    pwm_d = nc.declare_dram_parameter("pwm", [BPC, Lc, Lp], F32, isOutput=False)
    wc_d = nc.declare_dram_parameter("wc", [HID, 128], F32, isOutput=False)
    wp_d = nc.declare_dram_parameter("wp", [HID, 128], F32, isOutput=False)
    bc_d = nc.declare_dram_parameter("bc", [128], F32, isOutput=False)
    bp_d = nc.declare_dram_parameter("bp", [128], F32, isOutput=False)
    id_d = nc.declare_dram_parameter("ident", [128, 128], F32, isOutput=False)
    opair_d = nc.declare_dram_parameter("opair", [BPC, Lc, Lp], F32, isOutput=True)
    olat_d = nc.declare_dram_parameter("olat", [BPC, 2 * PROJ], F32, isOutput=True)

    with tile.TileContext(nc) as tc:
        with tc.tile_pool(name="cst", bufs=1) as cst, \
             tc.tile_pool(name="sb", bufs=2) as sb, \
             tc.tile_pool(name="sb3", bufs=3) as sb3, \
             tc.tile_pool(name="sb4", bufs=4) as sb4, \
             tc.tile_pool(name="one", bufs=1) as one, \
             tc.tile_pool(name="ps3", bufs=3, space="PSUM") as ps3, \
             tc.tile_pool(name="ps2", bufs=2, space="PSUM") as ps2, \
             tc.tile_pool(name="ps1", bufs=1, space="PSUM") as ps1:

            # ---- one-time constants (HWDGE + on-chip rounding; keeps the
            # SWDGE queue free for the first activation loads) ----
            ident_f = cst.tile([128, 128], F32, tag="identf")
            nc.sync.dma_start(ident_f[:], id_d[:])
            ident_r = cst.tile([128, 128], F16, tag="identr")
            nc.vector.tensor_copy(ident_r[:], ident_f[:])
            ones_col = cst.tile([128, 1], F16, tag="ones")
            nc.vector.memset(ones_col[:], 1.0)

            w_r = {}
            bias = {}
            for key, wd, bd in (("p", wp_d, bp_d), ("c", wc_d, bc_d)):
                wf = cst.tile([128, KT, 128], F32, tag="wstage")
                nc.sync.dma_start(wf[:], wd.rearrange("(t p) f -> p t f", p=128))
                wr = cst.tile([128, KT, 128], F16, tag=f"w{key}r")
                nc.vector.tensor_copy(wr[:], wf[:])
                w_r[key] = wr
                bt = cst.tile([128, 1], F32, tag=f"b{key}")
                nc.sync.dma_start(bt[:], bd[:, None])
                bias[key] = bt

            # staging for the four latent vectors [1, BPC*128]
            lat_stage = cst.tile([1, BPC * 128], F32, tag="latstage")

            cw = [0]  # round-robin DVE/ACT for PSUM->SBUF copies

            def transpose_chunk(xsrc_ap, n_sub, xt_tile):
                for k in range(KT):
                    tp = ps3.tile([128, n_sub * 128], F16, tag="tp")
                    for q in range(n_sub):
                        nc.tensor.transpose(
                            tp[:, q * 128:(q + 1) * 128],
                            xsrc_ap[:, q, k * 128:(k + 1) * 128], ident_r[:])
                    if cw[0] % 2 == 0:
                        nc.vector.tensor_copy(xt_tile[:, k, :], tp[:])
                    else:
                        nc.scalar.copy(xt_tile[:, k, :], tp[:])
                    cw[0] += 1

            # =========== PASS 1: projections (ACT does Gelu + copies) =====
            pmb_t, pwm_t, cmcol_t, projmp_t, latp_t = {}, {}, {}, {}, {}
            for b in range(BPC):
                # protein projections: projm_p [128(f_pw|f_lat), 1024]
                projm_p = sb4.tile([128, Lp], F16, tag="projmp")
                projmp_t[b] = projm_p
                # masks: tiny row loads + on-chip partition broadcast
                pmb = sb4.tile([128, Lp], F32, tag="pmb")
                nc.sync.dma_start(pmb[:], pm_d[b][None, :].partition_broadcast(128))
                pmb_t[b] = pmb
                cm_col = sb4.tile([128, 1], F32, tag="cmcol")
                nc.sync.dma_start(cm_col[:], cm_d[b][:, None])
                cmcol_t[b] = cm_col

                for ch in range(NPC):
                    l0 = ch * LPC
                    xn = sb.tile([128, 4, HID], F16, tag="xnat")
                    src = xp_d[b].rearrange("(c q p) d -> p c q d", p=128, q=4)
                    # split the load so transposes can start on the first half
                    nc.gpsimd.dma_start(xn[:, 0:2, :], src[:, ch, 0:2, :])
                    nc.gpsimd.dma_start(xn[:, 2:4, :], src[:, ch, 2:4, :])
                    xt = sb3.tile([128, KT, LPC], F16, tag="xt")
                    transpose_chunk(xn, 4, xt)
                    pj = ps2.tile([128, LPC], F32, tag="proj")
                    for k in range(KT):
                        nc.tensor.matmul(pj[:], w_r["p"][:, k, :], xt[:, k, :],
                                         start=(k == 0), stop=(k == KT - 1))
                    gl = sb.tile([128, LPC], F32, tag="gelu")
                    nc.scalar.activation(gl[:], pj[:], AF.Gelu, bias=bias["p"][:])
                    nc.vector.tensor_mul(projm_p[:, l0:l0 + LPC], gl[:], pmb[:, l0:l0 + LPC])

                # lat_p natural [128, 8, 64]
                lat_p = sb4.tile([128, 8, PROJ], F16, tag="latp")
                latp_t[b] = lat_p
                for t in range(8):
                    ltt = ps3.tile([128, 128], F16, tag="tp")
                    nc.tensor.transpose(
                        ltt[:], projm_p[:, t * 128:(t + 1) * 128], ident_r[:])
                    nc.scalar.copy(lat_p[:, t, :], ltt[:, PROJ:128])

                if b == 0:
                    # compound projections, all 4 samples batched: [128, 512]
                    xnc = one.tile([128, BPC, HID], F16, tag="xnatc")
                    nc.gpsimd.dma_start(xnc[:], xc_d.rearrange("b p d -> p b d"))
                    xtc = one.tile([128, KT, BPC * Lc], F16, tag="xtc")
                    transpose_chunk(xnc, BPC, xtc)
                    pjc = ps2.tile([128, BPC * Lc], F32, tag="proj")
                    for k in range(KT):
                        nc.tensor.matmul(pjc[:], w_r["c"][:, k, :], xtc[:, k, :],
                                         start=(k == 0), stop=(k == KT - 1))
                    glc = sb.tile([128, BPC * Lc], F32, tag="geluc")
                    nc.scalar.activation(glc[:], pjc[:], AF.Gelu, bias=bias["c"][:])
                    cmb = sb.tile([128, BPC * Lc], F32, tag="cmb")
                    nc.sync.dma_start(
                        cmb[:], cm_d.rearrange("b l -> (b l)")[None, :].partition_broadcast(128))
                    projm_c = sb.tile([128, BPC * Lc], F16, tag="projmc")
                    nc.vector.tensor_mul(projm_c[:], glc[:], cmb[:])

                    # lat_c natural [128, 64] per sample
                    lat_c = sb.tile([128, BPC, PROJ], F16, tag="latc")
                    for bb_ in range(BPC):
                        ltp = ps3.tile([128, 128], F16, tag="tp")
                        nc.tensor.transpose(
                            ltp[:], projm_c[:, bb_ * 128:(bb_ + 1) * 128], ident_r[:])
                        nc.vector.tensor_copy(lat_c[:, bb_, :], ltp[:, PROJ:128])

            # =========== PASS 2: pairwise + reductions (ACT does Sigmoid) ==
            for b in range(BPC):
                projm_p, pmb = projmp_t[b], pmb_t[b]
                cm_col, lat_p = cmcol_t[b], latp_t[b]
                pwm_sb = sb.tile([128, Lp], F32, tag="pwm")
                nc.sync.dma_start(pwm_sb[:], pwm_d[b])
                P_sb = sb.tile([128, Lp], F16, tag="P")
                outp = sb.tile([128, Lp], F32, tag="outp")
                for ch in range(NPC):
                    l0 = ch * LPC
                    mm = ps2.tile([128, LPC], F32, tag="mm")
                    nc.tensor.matmul(mm[:], projm_c[0:PROJ, b * 128:(b + 1) * 128],
                                     projm_p[0:PROJ, l0:l0 + LPC])
                    sg = sb.tile([128, LPC], F32, tag="sig")
                    nc.scalar.activation(sg[:], mm[:], AF.Sigmoid)
                    nc.vector.tensor_mul(P_sb[:, l0:l0 + LPC], mm[:], pwm_sb[:, l0:l0 + LPC])
                    nc.vector.tensor_mul(sg[:], sg[:], pmb[:, l0:l0 + LPC])
                    nc.vector.tensor_scalar_mul(outp[:, l0:l0 + LPC], sg[:], cm_col[:])
                nc.sync.dma_start(opair_d[b], outp[:])

                row = sb.tile([128, 1], F16, tag="row")
                with nc.allow_low_precision(reason="fp16 row sums, 2e-2 gate"):
                    nc.vector.reduce_sum(row[:], P_sb[:], axis=mybir.AxisListType.X)
                colt_ps = ps1.tile([128, 8], F32, tag="small")
                for t in range(8):
                    nc.tensor.matmul(colt_ps[:, t:t + 1],
                                     P_sb[:, t * 128:(t + 1) * 128], ones_col[:])
                colt = sb.tile([128, 8], F16, tag="coltsb")
                nc.vector.tensor_copy(colt[:], colt_ps[:])

                lv = ps1.tile([1, 2 * PROJ], F32, tag="small")
                nc.tensor.matmul(lv[:, 0:PROJ], row[:], lat_c[:, b, :])
                for t in range(8):
                    nc.tensor.matmul(lv[:, PROJ:2 * PROJ], colt[:, t:t + 1],
                                     lat_p[:, t, :], start=(t == 0), stop=(t == 7))
                nc.vector.tensor_copy(lat_stage[:, b * 128:(b + 1) * 128], lv[:])

            nc.sync.dma_start(olat_d.rearrange("b f -> (b f)")[None, :], lat_stage[:])

    _spill_excess_waits(nc)
    return nc


_NC_CACHE = [None]


def kernel(protein_features, pocket_mask, compound_features, compound_mask,
           pairwise_mask, W_pc, b_pc, W_pp, b_pp, W_lc, b_lc, W_lp, b_lp):
    import concourse.bass as bass
    from concourse.bass_utils import run_bass_kernel_spmd

    if _NC_CACHE[0] is None:
        _NC_CACHE[0] = _build(bass.Bass())
    nc = _NC_CACHE[0]

    f32 = np.float32
    wc = np.ascontiguousarray(np.concatenate([W_pc, W_lc], axis=1), dtype=f32)
    wp = np.ascontiguousarray(np.concatenate([W_pp, W_lp], axis=1), dtype=f32)
    bc = np.ascontiguousarray(np.concatenate([b_pc, b_lc]), dtype=f32)
    bp = np.ascontiguousarray(np.concatenate([b_pp, b_lp]), dtype=f32)
    ident = np.eye(128, dtype=f32)

    in_maps = []
    for c in range(NCORES):
        s = slice(c * BPC, (c + 1) * BPC)
        in_maps.append({
            "xp": np.ascontiguousarray(protein_features[s], dtype=f32),
            "xc": np.ascontiguousarray(compound_features[s], dtype=f32),
            "pm": np.ascontiguousarray(pocket_mask[s], dtype=f32),
            "cm": np.ascontiguousarray(compound_mask[s], dtype=f32),
            "pwm": np.ascontiguousarray(pairwise_mask[s], dtype=f32),
            "wc": wc, "wp": wp, "bc": bc, "bp": bp, "ident": ident,
        })

    res = run_bass_kernel_spmd(nc, in_maps, core_ids=list(range(NCORES)))
    latent = np.concatenate([res.results[c]["olat"] for c in range(NCORES)], axis=0)
    pair = np.concatenate([res.results[c]["opair"] for c in range(NCORES)], axis=0)
    return latent, pair
